# revision 1
# baseline (speedup 1.0000x reference)
"""Trainium2 Bass kernel for nn_NeuralAttention (cross-attention with RoPE).

Sharding: 8 cores = 4 batches (data parallel) x 2 head-groups (tensor
parallel, 8 heads each).  AllReduce over TP pairs after the output
projection.

Per-core device program (SPMD, per-core data):
  phase 1: Q/K/V projections in transposed layout (channels on partitions),
           RoPE applied via a block rotate-half permutation matmul + DVE
           combine with gathered cos/sin tables (dma_gather transpose).
  phase 2: per head-pair: row-packed score matmuls (d=64 contraction),
           Exp on ScalarE with fused 1/8 scale + per-key mask bias,
           attn@V matmuls with an appended ones column producing the
           softmax denominator for free, then normalization.
  phase 3: output projection (K=64 per head) + bo/2, DMA to DRAM.
  phase 4: AllReduce over {2b, 2b+1} pairs.
"""

import numpy as np
import ml_dtypes

import concourse.bass as bass
import concourse.mybir as mybir
from concourse import bacc
import concourse.tile as tile
from concourse import library_config
from concourse.bass_utils import run_bass_kernel_spmd

B, L, T = 4, 512, 4096
HID, NH, HD = 1024, 16, 64
MAX_POS, BASE = 4096, 10000.0
G = 2                 # TP head groups
NHG = NH // G         # heads per group
C = NHG * HD          # channels per group = 512
NCORES = 8

F32 = mybir.dt.float32
F32R = mybir.dt.float32r
BF16 = mybir.dt.bfloat16
I16 = mybir.dt.int16
U8 = mybir.dt.uint8

MULT = None  # set after import
ADD = None

_BF = ml_dtypes.bfloat16


# ---------------------------------------------------------------- host prep
def _host_tables():
    inv_freq = 1.0 / BASE ** (np.arange(0, HD, 2, dtype=np.float32) / HD)
    t = np.arange(MAX_POS, dtype=np.float32)
    freqs = np.einsum('i,j->ij', t, inv_freq).astype(np.float32)
    emb = np.concatenate([freqs, freqs], axis=-1)          # [MAX_POS, HD]
    return np.cos(emb).astype(np.float32), np.sin(emb).astype(np.float32)


def _rot_perm2():
    # P: rotate_half as a linear map; P2 = blockdiag(P, P)  [128, 128]
    P = np.zeros((HD, HD), np.float32)
    for d in range(HD // 2):
        P[d, d + HD // 2] = -1.0
        P[d + HD // 2, d] = 1.0
    P2 = np.zeros((128, 128), np.float32)
    P2[:64, :64] = P
    P2[64:, 64:] = P
    return P2


# ---------------------------------------------------------------- bass build
_NC_CACHE = {}
OPTS = {"no_cc": False, "no_gather": False}


def _build_nc():
    global MULT, ADD
    MULT = mybir.AluOpType.mult
    ADD = mybir.AluOpType.add
    EXP = mybir.ActivationFunctionType.Exp

    nc = bacc.Bacc(None, target_bir_lowering=False)

    # -------- DRAM parameters (per-core data fed via in_maps)
    tgtT = nc.declare_dram_parameter("tgtT", [HID, T], BF16, isOutput=False)       # target[b].T
    latT = nc.declare_dram_parameter("latT", [HID, L], BF16, isOutput=False)       # latents[b].T
    wkT = nc.declare_dram_parameter("wkT", [HID, C], BF16, isOutput=False)         # Wk_g.T
    wvT = nc.declare_dram_parameter("wvT", [HID, C], BF16, isOutput=False)
    wqT = nc.declare_dram_parameter("wqT", [HID, C], BF16, isOutput=False)
    woT = nc.declare_dram_parameter("woT", [C, HID], BF16, isOutput=False)         # Wo.T rows of group
    pt2 = nc.declare_dram_parameter("pt2", [128, 128], BF16, isOutput=False)       # P2.T
    cosq = nc.declare_dram_parameter("cosq", [128, L], BF16, isOutput=False)       # replicated x2
    sinq = nc.declare_dram_parameter("sinq", [128, L], BF16, isOutput=False)
    tabcs = nc.declare_dram_parameter("tabcs", [MAX_POS, 256], BF16, isOutput=False)  # [cos|cos|sin|sin]
    idx32 = nc.declare_dram_parameter("idx32", [128, T // 128], mybir.dt.int32, isOutput=False)
    eye = nc.declare_dram_parameter("eye", [128, 128], BF16, isOutput=False)
    if OPTS["no_gather"]:
        coskg = nc.declare_dram_parameter("coskg", [128, T], BF16, isOutput=False)
        sinkg = nc.declare_dram_parameter("sinkg", [128, T], BF16, isOutput=False)
    maskw = nc.declare_dram_parameter("maskw", [128, T // 128], U8, isOutput=False)
    bkw = nc.declare_dram_parameter("bkw", [128, C // 128], F32, isOutput=False)
    bqw = nc.declare_dram_parameter("bqw", [128, C // 128], F32, isOutput=False)
    bvrep = nc.declare_dram_parameter("bvrep", [128, C], F32, isOutput=False)
    borep = nc.declare_dram_parameter("borep", [128, HID], F32, isOutput=False)   # bo/2 replicated

    out = nc.declare_dram_parameter("out", [L, HID], F32, isOutput=True)
    cc_in = nc.dram_tensor("cc_in", [L, HID], F32)
    cc_out = nc.dram_tensor("cc_out", [L, HID], F32)

    TS = 512                    # t-slice width for phase 1
    NSL = T // TS               # 16 slices

    def mmr(out_ap, lhsT, rhs, **kw):
        nc.tensor.matmul(out_ap, lhsT, rhs, **kw)

    with tile.TileContext(nc) as tc:
        with tc.tile_pool(name="persist", bufs=1) as persist:
            # persistent across phases
            kpr = [persist.tile([128, T], BF16, tag=f"kpr{i}", name=f"kpr{i}")
                   for i in range(2)]
            qpr = [persist.tile([128, L], BF16, tag=f"qpr{i}", name=f"qpr{i}")
                   for i in range(4)]
            v_sb = persist.tile([128, T // 128, NHG, HD + 1], BF16, tag="v_sb")
            hT = persist.tile([64, NHG, L], BF16, tag="hT")
            ones_sb = persist.tile([128, 64], F32, tag="ones")
            mb_sb = persist.tile([128, T // 128], F32, tag="mb")

            nc.vector.memset(ones_sb[64:65, :], 1.0)
            # ones column of v (per head)
            nc.vector.memset(v_sb[:, :, :, HD:HD + 1], 1.0)

            # mask -> additive bias ( (m-1)*30000 : 0 keep, -30000 drop )
            with tc.tile_pool(name="mprep", bufs=1) as mprep:
                mk_sb = mprep.tile([128, T // 128], U8, tag="mk")
                nc.sync.dma_start(out=mk_sb, in_=maskw[:, :])
                nc.vector.tensor_copy(out=mb_sb, in_=mk_sb)       # u8 -> f32
                nc.vector.tensor_scalar_add(mb_sb, mb_sb, -1.0)
                nc.vector.tensor_scalar_mul(mb_sb, mb_sb, 30000.0)

            pt2_sb = persist.tile([128, 128], BF16, tag="pt2")
            nc.sync.dma_start(out=pt2_sb, in_=pt2[:, :])

            # ============== phase A: tables, Q proj, V proj (+ pair-0 K')
            tgc_cm = tc.tile_pool(name="tgc", bufs=1)
            tgc = tgc_cm.__enter__()
            tgT_sb = tgc.tile([128, 8, T], BF16, tag="tgT")
            for s in range(8):
                nc.sync.dma_start(
                    out=tgT_sb[:, :, s * 512:(s + 1) * 512],
                    in_=tgtT[:, s * 512:(s + 1) * 512].rearrange(
                        "(k p) t -> p k t", p=128))

            ph1c_cm = tc.tile_pool(name="ph1c", bufs=1)
            ph1c = ph1c_cm.__enter__()
            wk_sb = ph1c.tile([128, 8, C], BF16, tag="wk")
            nc.sync.dma_start(out=wk_sb, in_=wkT[:, :].rearrange("(k p) c -> p k c", p=128))
            wv_sb = ph1c.tile([128, 8, C], BF16, tag="wv")
            nc.sync.dma_start(out=wv_sb, in_=wvT[:, :].rearrange("(k p) c -> p k c", p=128))
            bk_sb = ph1c.tile([128, C // 128], F32, tag="bk")
            nc.sync.dma_start(out=bk_sb, in_=bkw[:, :])
            bv_sb = ph1c.tile([128, C], F32, tag="bv")
            nc.sync.dma_start(out=bv_sb, in_=bvrep[:, :])
            cosk_sb = ph1c.tile([128, 1, T], BF16, tag="cosk")
            sink_sb = ph1c.tile([128, 1, T], BF16, tag="sink")

            scr_cm = tc.tile_pool(name="scr", bufs=3)
            scr = scr_cm.__enter__()

            # issue the timestamp gathers up-front on the gpsimd queue;
            # the PE-transposes that consume them are interleaved into the
            # V-projection loop below so the PE never waits on them.
            gth_cm = tc.tile_pool(name="gth", bufs=1)
            gth = gth_cm.__enter__()
            tcs_sb = None
            if OPTS["no_gather"]:
                nc.sync.dma_start(out=cosk_sb[:, 0, :], in_=coskg[:, :])
                nc.sync.dma_start(out=sink_sb[:, 0, :], in_=sinkg[:, :])
            else:
                idx_sb = gth.tile([128, T // 128], mybir.dt.int32, tag="idx")
                nc.sync.dma_start(out=idx_sb, in_=idx32[:, :])
                eye_sb = gth.tile([128, 128], BF16, tag="eye")
                nc.sync.dma_start(out=eye_sb, in_=eye[:, :])
                tcs_sb = gth.tile([128, T // 128, 256], BF16, tag="tcs")
                for tt in range(T // 128):
                    nc.gpsimd.indirect_dma_start(
                        out=tcs_sb[:, tt, :], out_offset=None,
                        in_=tabcs[:, :],
                        in_offset=bass.IndirectOffsetOnAxis(
                            ap=idx_sb[:, tt:tt + 1], axis=0))

            # ---- Q projection + rope (first PE work; no table dependency)
            with tc.tile_pool(name="qc", bufs=1) as qc, \
                 tc.tile_pool(name="qps", bufs=2, space="PSUM") as qps, \
                 tc.tile_pool(name="qrp", bufs=2, space="PSUM") as qrp:
                wq_sb = qc.tile([128, 8, C], BF16, tag="wq")
                nc.sync.dma_start(out=wq_sb, in_=wqT[:, :].rearrange("(k p) c -> p k c", p=128))
                lat_sb = qc.tile([128, 8, L], BF16, tag="lat")
                nc.sync.dma_start(out=lat_sb, in_=latT[:, :].rearrange("(k p) l -> p k l", p=128))
                bq_sb = qc.tile([128, C // 128], F32, tag="bq")
                nc.sync.dma_start(out=bq_sb, in_=bqw[:, :])
                cq_sb = qc.tile([128, L], BF16, tag="cq")
                nc.sync.dma_start(out=cq_sb, in_=cosq[:, :])
                sq_sb = qc.tile([128, L], BF16, tag="sq")
                nc.sync.dma_start(out=sq_sb, in_=sinq[:, :])

                for ct in range(4):
                    qp = qps.tile([128, L], F32, tag="qp")
                    for k in range(8):
                        mmr(qp, wq_sb[:, k, ct * 128:(ct + 1) * 128],
                            lat_sb[:, k, :], start=(k == 0), stop=(k == 7))
                    qsb = scr.tile([128, L], BF16, tag="ksb")
                    nc.vector.tensor_scalar_add(qsb, qp, bq_sb[:, ct:ct + 1])
                    qr = qrp.tile([128, L], F32, tag="qr")
                    mmr(qr, pt2_sb, qsb, start=True, stop=True)
                    t1 = scr.tile([128, L], BF16, tag="t1")
                    nc.vector.tensor_tensor(t1, qsb, cq_sb, MULT)
                    t2 = scr.tile([128, L], BF16, tag="t2")
                    nc.vector.tensor_tensor(t2, qr, sq_sb, MULT)
                    nc.vector.tensor_tensor(qpr[ct], t1, t2, ADD)

            # ---- K' helpers (k' for pair pn into kdst, slice s of 512)
            kps_cm = tc.tile_pool(name="kps", bufs=1, space="PSUM")
            kps = kps_cm.__enter__()
            rps_cm = tc.tile_pool(name="rps", bufs=1, space="PSUM")
            rps = rps_cm.__enter__()

            def emit_kgroup(pn, s):
                kp = kps.tile([128, 512], F32, tag="kp", name="kp")
                for k in range(8):
                    mmr(kp, wk_sb[:, k, pn * 128:(pn + 1) * 128],
                        tgT_sb[:, k, s * 512:(s + 1) * 512],
                        start=(k == 0), stop=(k == 7))
                ksb = scr.tile([128, 512], BF16, tag="ksb", name="ksb")
                nc.vector.tensor_scalar_add(ksb, kp, bk_sb[:, pn:pn + 1])
                return ksb

            def emit_krope(s, ksb, kdst):
                kr = rps.tile([128, 512], F32, tag="kr", name="kr")
                mmr(kr, pt2_sb, ksb, start=True, stop=True)
                t1 = scr.tile([128, 512], BF16, tag="t1", name="t1")
                nc.vector.tensor_tensor(t1, ksb, cosk_sb[:, 0, s * 512:(s + 1) * 512], MULT)
                t2 = scr.tile([128, 512], BF16, tag="t2", name="t2")
                nc.vector.tensor_tensor(t2, kr, sink_sb[:, 0, s * 512:(s + 1) * 512], MULT)
                nc.vector.tensor_tensor(kdst[:, s * 512:(s + 1) * 512], t1, t2, ADD)

            # ---- V projection with table transposes + pair-0 K' interleaved
            with tc.tile_pool(name="vps", bufs=2, space="PSUM") as vps, \
                 tc.tile_pool(name="tps", bufs=2, space="PSUM") as tps:
                ksb_pend = None
                for tt in range(T // 128):
                    vp = vps.tile([128, C], F32, tag="vp")
                    for k in range(8):
                        mmr(vp, tgT_sb[:, k, tt * 128:(tt + 1) * 128],
                            wv_sb[:, k, :], start=(k == 0), stop=(k == 7))
                    nc.vector.tensor_tensor(
                        v_sb[:, tt, :, 0:HD],
                        vp.rearrange("p (h d) -> p h d", h=NHG),
                        bv_sb.rearrange("p (h d) -> p h d", h=NHG), ADD)
                    if tcs_sb is not None:
                        tpc = tps.tile([128, 128], BF16, tag="tp", name="tpc")
                        nc.tensor.transpose(out=tpc, in_=tcs_sb[:, tt, 0:128],
                                            identity=eye_sb)
                        nc.vector.tensor_copy(
                            out=cosk_sb[:, 0, tt * 128:(tt + 1) * 128], in_=tpc)
                        tpsn = tps.tile([128, 128], BF16, tag="tp", name="tpsn")
                        nc.tensor.transpose(out=tpsn, in_=tcs_sb[:, tt, 128:256],
                                            identity=eye_sb)
                        nc.vector.tensor_copy(
                            out=sink_sb[:, 0, tt * 128:(tt + 1) * 128], in_=tpsn)
                    if tt % 4 == 0:
                        ksb_pend = emit_kgroup(0, tt // 4)
                    elif tt % 4 == 3:
                        emit_krope(tt // 4, ksb_pend, kpr[0])
            gth_cm.__exit__(None, None, None)

            # ===== phase B: per-pair attention, next pair's K' interleaved
            sps_cm = tc.tile_pool(name="sps", bufs=2, space="PSUM")
            sps = sps_cm.__enter__()
            avp_cm = tc.tile_pool(name="avp", bufs=1, space="PSUM")
            avp = avp_cm.__enter__()
            escr_cm = tc.tile_pool(name="escr", bufs=3)
            escr = escr_cm.__enter__()
            scr2_cm = tc.tile_pool(name="scr2", bufs=2)
            scr2 = scr2_cm.__enter__()
            NT = T // 128     # 32 key tiles
            for p in range(4):
                hA, hB = 2 * p, 2 * p + 1
                kcur = kpr[p % 2]
                knext = kpr[(p + 1) % 2]
                avA = avp.tile([65, L], F32, tag="avA", name="avA")
                avB = avp.tile([65, L], F32, tag="avB", name="avB")
                es = {}
                ksb_pend = None
                for tt in range(NT):
                    sAB = sps.tile([128, 2, L], F32, tag="sAB", name="sAB")
                    nc.tensor.matmul(sAB[:, 0, :],
                                     kcur[0:64, tt * 128:(tt + 1) * 128],
                                     qpr[p][0:64, :], start=True, stop=True)
                    nc.tensor.matmul(sAB[:, 1, :],
                                     kcur[64:128, tt * 128:(tt + 1) * 128],
                                     qpr[p][64:128, :], start=True, stop=True)
                    eAB = escr.tile([128, 2, L], BF16, tag="eAB", name="eAB")
                    nc.scalar.activation(out=eAB, in_=sAB, func=EXP,
                                         bias=mb_sb[:, tt:tt + 1], scale=0.125)
                    es[tt] = eAB
                    if tt > 0:
                        eP = es.pop(tt - 1)
                        nc.tensor.matmul(avA, v_sb[:, tt - 1, hA, :], eP[:, 0, :],
                                         start=(tt - 1 == 0), stop=False)
                        nc.tensor.matmul(avB, v_sb[:, tt - 1, hB, :], eP[:, 1, :],
                                         start=(tt - 1 == 0), stop=False)
                    if p < 3:
                        if tt % 4 == 0:
                            ksb_pend = emit_kgroup(p + 1, tt // 4)
                        elif tt % 4 == 2:
                            emit_krope(tt // 4, ksb_pend, knext)
                eP = es.pop(NT - 1)
                nc.tensor.matmul(avA, v_sb[:, NT - 1, hA, :], eP[:, 0, :],
                                 start=False, stop=True)
                nc.tensor.matmul(avB, v_sb[:, NT - 1, hB, :], eP[:, 1, :],
                                 start=False, stop=True)

                for av, h in ((avA, hA), (avB, hB)):
                    dn = scr2.tile([128, L], F32, tag="dn", name="dn")
                    nc.vector.tensor_copy(out=dn[64:65, :], in_=av[64:65, :])
                    nc.vector.reciprocal(out=dn[64:65, :], in_=dn[64:65, :])
                    bc = sps.tile([64, L], F32, tag="sAB", name="bc")
                    nc.tensor.matmul(bc, ones_sb[64:65, :], dn[64:65, :],
                                     start=True, stop=True)
                    osb = scr2.tile([64, L], F32, tag="osb", name="osb")
                    nc.vector.tensor_copy(out=osb, in_=av[0:64, :])
                    nc.vector.tensor_tensor(hT[:, h, :], osb, bc, MULT)

            scr2_cm.__exit__(None, None, None)
            escr_cm.__exit__(None, None, None)
            avp_cm.__exit__(None, None, None)
            sps_cm.__exit__(None, None, None)
            rps_cm.__exit__(None, None, None)
            kps_cm.__exit__(None, None, None)
            scr_cm.__exit__(None, None, None)

            # =================================================== phase 3
            with tc.tile_pool(name="wop", bufs=4) as wop, \
                 tc.tile_pool(name="ops", bufs=1, space="PSUM") as ops, \
                 tc.tile_pool(name="ow", bufs=3) as ow:
                bo_sb = wop.tile([128, HID], F32, tag="bo")
                nc.sync.dma_start(out=bo_sb, in_=borep[:, :])
                for n in range(2):
                    opst = [ops.tile([128, 512], F32, tag=f"op{lt}", name=f"op{lt}")
                            for lt in range(4)]
                    for h in range(NHG):
                        wot = wop.tile([64, 512], BF16, tag="wot", name="wot")
                        nc.sync.dma_start(
                            out=wot,
                            in_=woT[h * 64:(h + 1) * 64, n * 512:(n + 1) * 512])
                        for lt in range(4):
                            mmr(opst[lt], hT[:, h, lt * 128:(lt + 1) * 128],
                                wot, start=(h == 0), stop=(h == NHG - 1))
                    for lt in range(4):
                        ob = ow.tile([128, 512], F32, tag="ob", name="ob")
                        nc.vector.tensor_tensor(
                            ob, opst[lt], bo_sb[:, n * 512:(n + 1) * 512], ADD)
                        nc.sync.dma_start(
                            out=cc_in[lt * 128:(lt + 1) * 128,
                                      n * 512:(n + 1) * 512],
                            in_=ob)
                if OPTS["no_cc"]:
                    nc.sync.dma_start(out=out[:, :], in_=cc_in[:, :])
                else:
                    nc.gpsimd.collective_compute(
                        "AllReduce", mybir.AluOpType.add,
                        ins=[cc_in[:, :]], outs=[cc_out[:, :]],
                        replica_groups=[[0, 1], [2, 3], [4, 5], [6, 7]],
                    )
                    nc.sync.dma_start(out=out[:, :], in_=cc_out[:, :])
            ph1c_cm.__exit__(None, None, None)
            tgc_cm.__exit__(None, None, None)

    return nc


def get_nc():
    key = tuple(sorted(OPTS.items()))
    if key not in _NC_CACHE:
        nc = _build_nc()
        if not nc.is_finalized():
            nc.finalize()
        _NC_CACHE[key] = nc
    return _NC_CACHE[key]


# ---------------------------------------------------------------- host side
def make_in_maps(latents, target, target_mask, target_timestamp,
                 Wq, bq, Wk, bk, Wv, bv, Wo, bo):
    cos_tab, sin_tab = _host_tables()
    P2 = _rot_perm2()

    lat_ts = (np.arange(L, dtype=np.float32) * (MAX_POS - 1) / (L - 1)).astype(np.int64)
    cosq = np.tile(cos_tab[lat_ts].T, (2, 1)).astype(_BF)   # [128, L]
    sinq = np.tile(sin_tab[lat_ts].T, (2, 1)).astype(_BF)

    tabcs = np.ascontiguousarray(np.concatenate(
        [cos_tab, cos_tab, sin_tab, sin_tab], axis=1)).astype(_BF)  # [4096, 256]

    WoT = np.ascontiguousarray(Wo.T)

    in_maps = []
    for core in range(NCORES):
        b, g = core // 2, core % 2
        sl = slice(g * C, (g + 1) * C)
        ts = np.asarray(target_timestamp[b]).astype(np.int64)
        idx_w = np.ascontiguousarray(ts.reshape(T // 128, 128).T.astype(np.int32))
        mask = np.asarray(target_mask[b]).astype(np.uint8)
        m = {
            "tgtT": np.ascontiguousarray(np.asarray(target[b]).T).astype(_BF),
            "latT": np.ascontiguousarray(np.asarray(latents[b]).T).astype(_BF),
            "wkT": np.ascontiguousarray(np.asarray(Wk)[sl, :].T).astype(_BF),
            "wvT": np.ascontiguousarray(np.asarray(Wv)[sl, :].T).astype(_BF),
            "wqT": np.ascontiguousarray(np.asarray(Wq)[sl, :].T).astype(_BF),
            "woT": np.ascontiguousarray(WoT[sl, :]).astype(_BF),
            "pt2": np.ascontiguousarray(P2.T).astype(_BF),
            "cosq": cosq, "sinq": sinq,
            "tabcs": tabcs,
            "idx32": idx_w,
            "eye": np.eye(128, dtype=_BF),
            "maskw": np.ascontiguousarray(mask.reshape(T // 128, 128).T),
            "bkw": np.ascontiguousarray(
                np.asarray(bk)[sl].reshape(C // 128, 128).T.astype(np.float32)),
            "bqw": np.ascontiguousarray(
                np.asarray(bq)[sl].reshape(C // 128, 128).T.astype(np.float32)),
            "bvrep": np.ascontiguousarray(
                np.tile(np.asarray(bv)[sl][None, :], (128, 1)).astype(np.float32)),
            "borep": np.ascontiguousarray(
                np.tile(0.5 * np.asarray(bo)[None, :], (128, 1)).astype(np.float32)),
        }
        if OPTS["no_gather"]:
            m["coskg"] = np.ascontiguousarray(tabcs[ts, 0:128].T)
            m["sinkg"] = np.ascontiguousarray(tabcs[ts, 128:256].T)
        in_maps.append(m)
    return in_maps


def kernel(latents, target, target_mask, target_timestamp,
           Wq, bq, Wk, bk, Wv, bv, Wo, bo, _trace=False, _trace_kwargs=None):
    in_maps = make_in_maps(latents, target, target_mask, target_timestamp,
                           Wq, bq, Wk, bk, Wv, bv, Wo, bo)
    nc = get_nc()
    res = run_bass_kernel_spmd(nc, in_maps, list(range(NCORES)),
                               trace=_trace, **(_trace_kwargs or {}))
    full = np.zeros((B, L, HID), np.float32)
    for b in range(B):
        if OPTS["no_cc"]:
            full[b] = res.results[2 * b]["out"] + res.results[2 * b + 1]["out"]
        else:
            full[b] = res.results[2 * b]["out"]
    if _trace:
        return full, res
    return full



# revision 9
# speedup vs baseline: 1.2384x; 1.2384x over previous
"""Trainium2 Bass kernel for nn_NeuralAttention (cross-attention with RoPE).

Sharding: 8 cores = 4 batches (data parallel) x 2 head-groups (tensor
parallel, 8 heads each).  Per-pair AllGather of the normalized per-head
attention outputs BEFORE the output projection; each core then computes
the full 16-head output projection for its half of the output columns
and writes that half directly (no AllReduce).

Per-core device program (SPMD, per-core data):
  phase A: Q projection + RoPE (starts ~5us; chunked weight/latent DMAs),
           V projection with pair-0 K'+RoPE interleaved.  All tables
           (cos/sin for keys) are host-gathered, no on-device gather.
  phase B: per head-pair: row-packed score matmuls (d=64 contraction,
           2x concurrent via PE row tiling), Exp on ScalarE with fused
           1/8 scale + per-key mask bias, attn@V matmuls with an appended
           ones column producing the softmax denominator for free;
           normalization via reciprocal_approx_fast + ones-broadcast
           matmul (off the score-PSUM pool).
  phase C: AllGather [my 8 heads] <-> pair core (512KB bf16), then
           output projection with K=128 head-pair-packed contraction,
           bias, DMA of the column half to DRAM.
"""

import numpy as np
import ml_dtypes

import concourse.bass as bass
import concourse.mybir as mybir
from concourse import bacc
import concourse.tile as tile
from concourse.bass_utils import run_bass_kernel_spmd

B, L, T = 4, 512, 4096
HID, NH, HD = 1024, 16, 64
MAX_POS, BASE = 4096, 10000.0
G = 2                 # TP head groups
NHG = NH // G         # heads per group
C = NHG * HD          # channels per group = 512
NCORES = 8
NSL = T // 512        # 8 key slices of 512
NT = T // 128         # 32 key tiles of 128

F32 = mybir.dt.float32
BF16 = mybir.dt.bfloat16

MULT = None
ADD = None

_BF = ml_dtypes.bfloat16


# ---------------------------------------------------------------- host prep
def _host_tables():
    inv_freq = 1.0 / BASE ** (np.arange(0, HD, 2, dtype=np.float32) / HD)
    t = np.arange(MAX_POS, dtype=np.float32)
    freqs = np.einsum('i,j->ij', t, inv_freq).astype(np.float32)
    emb = np.concatenate([freqs, freqs], axis=-1)          # [MAX_POS, HD]
    return np.cos(emb).astype(np.float32), np.sin(emb).astype(np.float32)


def _rot_perm2():
    # P: rotate_half as a linear map; P2 = blockdiag(P, P)  [128, 128]
    P = np.zeros((HD, HD), np.float32)
    for d in range(HD // 2):
        P[d, d + HD // 2] = -1.0
        P[d + HD // 2, d] = 1.0
    P2 = np.zeros((128, 128), np.float32)
    P2[:64, :64] = P
    P2[64:, 64:] = P
    return P2


# ---------------------------------------------------------------- bass build
_NC_CACHE = {}
OPTS = {"no_cc": False}


def _build_nc():
    global MULT, ADD
    MULT = mybir.AluOpType.mult
    ADD = mybir.AluOpType.add
    EXP = mybir.ActivationFunctionType.Exp

    nc = bacc.Bacc(None, target_bir_lowering=False)

    # -------- DRAM parameters (per-core data fed via in_maps)
    lat8 = nc.declare_dram_parameter("lat8", [8, 128, L], BF16, isOutput=False)
    wq8 = nc.declare_dram_parameter("wq8", [8, 128, C], BF16, isOutput=False)
    bqw = nc.declare_dram_parameter("bqw", [128, C // 128], F32, isOutput=False)
    cosq = nc.declare_dram_parameter("cosq", [128, L], BF16, isOutput=False)
    sinq = nc.declare_dram_parameter("sinq", [128, L], BF16, isOutput=False)
    pt2 = nc.declare_dram_parameter("pt2", [128, 128], BF16, isOutput=False)
    mbias = nc.declare_dram_parameter("mbias", [128, NT], F32, isOutput=False)

    wk1 = nc.declare_dram_parameter("wk1", [128, 8 * C], BF16, isOutput=False)
    bkw = nc.declare_dram_parameter("bkw", [128, C // 128], F32, isOutput=False)
    tgt8 = nc.declare_dram_parameter("tgt8", [NSL, 128, T], BF16, isOutput=False)
    wv1 = nc.declare_dram_parameter("wv1", [128, 8 * C], BF16, isOutput=False)
    bvrep = nc.declare_dram_parameter("bvrep", [128, C], F32, isOutput=False)
    cosk8 = nc.declare_dram_parameter("cosk8", [NSL, 128, 512], BF16, isOutput=False)
    sink8 = nc.declare_dram_parameter("sink8", [NSL, 128, 512], BF16, isOutput=False)

    wo2p = nc.declare_dram_parameter("wo2", [128, NHG * 512], BF16, isOutput=False)
    borep = nc.declare_dram_parameter("borep", [128, 512], F32, isOutput=False)

    out = nc.declare_dram_parameter("out", [L, 512], F32, isOutput=True)
    hx_in = nc.dram_tensor("hx_in", [64, NHG * L], BF16)
    hx_out = nc.dram_tensor("hx_out", [128, NHG * L], BF16)

    def mmr(out_ap, lhsT, rhs, **kw):
        nc.tensor.matmul(out_ap, lhsT, rhs, **kw)

    with tile.TileContext(nc) as tc:
        with tc.tile_pool(name="persist", bufs=1) as persist:
            # persistent tiles
            kpr = [persist.tile([128, T], BF16, tag=f"kpr{i}", name=f"kpr{i}")
                   for i in range(2)]
            qpr = [persist.tile([128, L], BF16, tag=f"qpr{i}", name=f"qpr{i}")
                   for i in range(4)]
            v_sb = persist.tile([128, NT, NHG, HD + 1], BF16, tag="v_sb")
            hT2 = persist.tile([128, NHG, L], BF16, tag="hT2")
            ones_sb = persist.tile([128, 64], F32, tag="ones")
            mb_sb = persist.tile([128, NT], F32, tag="mb")
            pt2_sb = persist.tile([128, 128], BF16, tag="pt2")
            wk_sb = persist.tile([128, 8, C], BF16, tag="wk")
            bk_sb = persist.tile([128, C // 128], F32, tag="bk")
            tg2 = persist.tile([128, NSL, 8, 512], BF16, tag="tg2")
            wv_sb = persist.tile([128, 8, C], BF16, tag="wv")
            bv_sb = persist.tile([128, C], F32, tag="bv")
            cosk_sb = persist.tile([128, T], BF16, tag="cosk")
            sink_sb = persist.tile([128, T], BF16, tag="sink")
            wo2_sb = persist.tile([128, NHG, 512], BF16, tag="wo2")
            bo_sb = persist.tile([128, 512], F32, tag="bo")

            scr_cm = tc.tile_pool(name="scr", bufs=3)
            scr = scr_cm.__enter__()
            qc_cm = tc.tile_pool(name="qc", bufs=1)
            qc = qc_cm.__enter__()
            lat_sb = qc.tile([128, 8, L], BF16, tag="lat")
            wq_sb = qc.tile([128, 8, C], BF16, tag="wq")
            bq_sb = qc.tile([128, C // 128], F32, tag="bq")
            cq_sb = qc.tile([128, L], BF16, tag="cq")
            sq_sb = qc.tile([128, L], BF16, tag="sq")

            # ---- all input DMAs issued up front, in priority order
            # (Q-projection dependencies first so the PE can start ~5us in)
            for k in range(8):
                nc.sync.dma_start(out=lat_sb[:, k, :], in_=lat8[k, :, :])
                nc.sync.dma_start(out=wq_sb[:, k, :], in_=wq8[k, :, :])
            nc.sync.dma_start(out=bq_sb, in_=bqw[:, :])
            nc.sync.dma_start(out=cq_sb, in_=cosq[:, :])
            nc.sync.dma_start(out=sq_sb, in_=sinq[:, :])
            nc.sync.dma_start(out=pt2_sb, in_=pt2[:, :])
            nc.sync.dma_start(out=mb_sb, in_=mbias[:, :])
            nc.sync.dma_start(
                out=wk_sb, in_=wk1[:, :].rearrange("p (k c) -> p k c", k=8))
            nc.sync.dma_start(out=bk_sb, in_=bkw[:, :])
            nc.sync.dma_start(
                out=tg2[:, 0, :, :],
                in_=tgt8[0, :, :].rearrange("p (k t) -> p k t", k=8))
            nc.sync.dma_start(
                out=wv_sb, in_=wv1[:, :].rearrange("p (k c) -> p k c", k=8))
            nc.sync.dma_start(out=bv_sb, in_=bvrep[:, :])
            for s in range(NSL):
                if s > 0:
                    nc.sync.dma_start(
                        out=tg2[:, s, :, :],
                        in_=tgt8[s, :, :].rearrange("p (k t) -> p k t", k=8))
                nc.sync.dma_start(
                    out=cosk_sb[:, s * 512:(s + 1) * 512], in_=cosk8[s, :, :])
                nc.sync.dma_start(
                    out=sink_sb[:, s * 512:(s + 1) * 512], in_=sink8[s, :, :])
            nc.sync.dma_start(
                out=wo2_sb, in_=wo2p[:, :].rearrange("p (h c) -> p h c", h=NHG))
            nc.sync.dma_start(out=bo_sb, in_=borep[:, :])

            nc.vector.memset(ones_sb[64:65, :], 1.0)
            nc.vector.memset(v_sb[:, :, :, HD:HD + 1], 1.0)

            # ---- Q projection + rope (first PE work)
            with tc.tile_pool(name="qps", bufs=2, space="PSUM") as qps, \
                 tc.tile_pool(name="qrp", bufs=2, space="PSUM") as qrp:
                for ct in range(4):
                    qp = qps.tile([128, L], F32, tag="qp")
                    for k in range(8):
                        mmr(qp, wq_sb[:, k, ct * 128:(ct + 1) * 128],
                            lat_sb[:, k, :], start=(k == 0), stop=(k == 7))
                    qsb = scr.tile([128, L], BF16, tag="ksb")
                    nc.vector.tensor_scalar_add(qsb, qp, bq_sb[:, ct:ct + 1])
                    qr = qrp.tile([128, L], F32, tag="qr")
                    mmr(qr, pt2_sb, qsb, start=True, stop=True)
                    t1 = scr.tile([128, L], BF16, tag="t1")
                    nc.vector.tensor_tensor(t1, qsb, cq_sb, MULT)
                    t2 = scr.tile([128, L], BF16, tag="t2")
                    nc.vector.tensor_tensor(t2, qr, sq_sb, MULT)
                    nc.vector.tensor_tensor(qpr[ct], t1, t2, ADD)
            qc_cm.__exit__(None, None, None)

            # ---- K' helpers (k' for pair pn, slice s of 512)
            kps_cm = tc.tile_pool(name="kps", bufs=1, space="PSUM")
            kps = kps_cm.__enter__()
            rps_cm = tc.tile_pool(name="rps", bufs=1, space="PSUM")
            rps = rps_cm.__enter__()

            def emit_kgroup(pn, s):
                kp = kps.tile([128, 512], F32, tag="kp", name="kp")
                for k in range(8):
                    mmr(kp, wk_sb[:, k, pn * 128:(pn + 1) * 128],
                        tg2[:, s, k, :], start=(k == 0), stop=(k == 7))
                ksb = scr.tile([128, 512], BF16, tag="ksb", name="ksb")
                nc.vector.tensor_scalar_add(ksb, kp, bk_sb[:, pn:pn + 1])
                return ksb

            def emit_krope(s, ksb, kdst):
                kr = rps.tile([128, 512], F32, tag="kr", name="kr")
                mmr(kr, pt2_sb, ksb, start=True, stop=True)
                t1 = scr.tile([128, 512], BF16, tag="t1", name="t1")
                nc.vector.tensor_tensor(t1, ksb, cosk_sb[:, s * 512:(s + 1) * 512], MULT)
                t2 = scr.tile([128, 512], BF16, tag="t2", name="t2")
                nc.vector.tensor_tensor(t2, kr, sink_sb[:, s * 512:(s + 1) * 512], MULT)
                nc.vector.tensor_tensor(kdst[:, s * 512:(s + 1) * 512], t1, t2, ADD)

            # ---- V projection with pair-0 K' interleaved
            with tc.tile_pool(name="vps", bufs=2, space="PSUM") as vps:
                ksb_pend = None
                for tt in range(NT):
                    if tt % 4 == 0:
                        ksb_pend = emit_kgroup(0, tt // 4)
                    vp = vps.tile([128, C], F32, tag="vp")
                    for k in range(8):
                        mmr(vp, tg2[:, tt // 4, k, (tt % 4) * 128:(tt % 4 + 1) * 128],
                            wv_sb[:, k, :], start=(k == 0), stop=(k == 7))
                    nc.vector.tensor_tensor(
                        v_sb[:, tt, :, 0:HD],
                        vp.rearrange("p (h d) -> p h d", h=NHG),
                        bv_sb.rearrange("p (h d) -> p h d", h=NHG), ADD)
                    if tt % 4 == 3:
                        emit_krope(tt // 4, ksb_pend, kpr[0])

            # ===== phase B: per-pair attention, next pair's K' interleaved
            sps_cm = tc.tile_pool(name="sps", bufs=2, space="PSUM")
            sps = sps_cm.__enter__()
            avp_cm = tc.tile_pool(name="avp", bufs=1, space="PSUM")
            avp = avp_cm.__enter__()
            escr_cm = tc.tile_pool(name="escr", bufs=3)
            escr = escr_cm.__enter__()
            scr2_cm = tc.tile_pool(name="scr2", bufs=2)
            scr2 = scr2_cm.__enter__()
            for p in range(4):
                hA, hB = 2 * p, 2 * p + 1
                kcur = kpr[p % 2]
                knext = kpr[(p + 1) % 2]
                avA = avp.tile([65, L], F32, tag="avA", name="avA")
                avB = avp.tile([65, L], F32, tag="avB", name="avB")
                es = {}
                ksb_pend = None
                for tt in range(NT):
                    sAB = sps.tile([128, 2, L], F32, tag="sAB", name="sAB")
                    nc.tensor.matmul(sAB[:, 0, :],
                                     kcur[0:64, tt * 128:(tt + 1) * 128],
                                     qpr[p][0:64, :], start=True, stop=True)
                    nc.tensor.matmul(sAB[:, 1, :],
                                     kcur[64:128, tt * 128:(tt + 1) * 128],
                                     qpr[p][64:128, :], start=True, stop=True)
                    eAB = escr.tile([128, 2, L], BF16, tag="eAB", name="eAB")
                    nc.scalar.activation(out=eAB, in_=sAB, func=EXP,
                                         bias=mb_sb[:, tt:tt + 1], scale=0.125)
                    es[tt] = eAB
                    if tt > 0:
                        eP = es.pop(tt - 1)
                        nc.tensor.matmul(avA, v_sb[:, tt - 1, hA, :], eP[:, 0, :],
                                         start=(tt - 1 == 0), stop=False)
                        nc.tensor.matmul(avB, v_sb[:, tt - 1, hB, :], eP[:, 1, :],
                                         start=(tt - 1 == 0), stop=False)
                    if p < 3:
                        if tt % 4 == 0:
                            ksb_pend = emit_kgroup(p + 1, tt // 4)
                        elif tt % 4 == 2:
                            emit_krope(tt // 4, ksb_pend, knext)
                eP = es.pop(NT - 1)
                nc.tensor.matmul(avA, v_sb[:, NT - 1, hA, :], eP[:, 0, :],
                                 start=False, stop=True)
                nc.tensor.matmul(avB, v_sb[:, NT - 1, hB, :], eP[:, 1, :],
                                 start=False, stop=True)

                # normalization: denominator is row 64 of av; broadcast its
                # reciprocal over the 64 output rows via a ones-column matmul
                # allocated from rps (NOT sps -- keeps next pair's scores
                # independent of this chain).
                for av, h in ((avA, hA), (avB, hB)):
                    dn = scr2.tile([128, L], F32, tag="dn", name="dn")
                    nc.vector.tensor_copy(out=dn[64:65, :], in_=av[64:65, :])
                    nc.vector.reciprocal(
                        out=dn[64:65, :], in_=dn[64:65, :])
                    osb = scr2.tile([64, L], BF16, tag="osb", name="osb")
                    nc.vector.tensor_copy(out=osb, in_=av[0:64, :])
                    bc = rps.tile([128, 512], F32, tag="kr", name="bc")
                    nc.tensor.matmul(bc[0:64, :], ones_sb[64:65, :],
                                     dn[64:65, :], start=True, stop=True)
                    nc.vector.tensor_tensor(hT2[0:64, h, :], osb, bc[0:64, :], MULT)

            scr2_cm.__exit__(None, None, None)
            escr_cm.__exit__(None, None, None)
            avp_cm.__exit__(None, None, None)
            sps_cm.__exit__(None, None, None)
            rps_cm.__exit__(None, None, None)
            kps_cm.__exit__(None, None, None)
            scr_cm.__exit__(None, None, None)

            # ===== phase C: exchange heads with pair core, output projection
            nc.sync.dma_start(
                out=hx_in[:, :],
                in_=hT2[0:64, :, :].rearrange("p h l -> p (h l)"))
            if OPTS["no_cc"]:
                nc.sync.dma_start(
                    out=hT2[:, :, :],
                    in_=hx_in[:, :].rearrange("p (h l) -> p h l", h=NHG))
                nc.sync.dma_start(
                    out=hT2[64:128, :, :],
                    in_=hx_in[:, :].rearrange("p (h l) -> p h l", h=NHG))
            else:
                nc.gpsimd.collective_compute(
                    "AllGather", mybir.AluOpType.bypass,
                    ins=[hx_in[:, :]], outs=[hx_out[:, :]],
                    replica_groups=[[0, 1], [2, 3], [4, 5], [6, 7]],
                )
                # rows 0:64 = g0 chunk, 64:128 = g1 chunk (fixed layout on
                # both cores; own data round-trips through DRAM)
                nc.sync.dma_start(
                    out=hT2[:, :, :],
                    in_=hx_out[:, :].rearrange("p (h l) -> p h l", h=NHG))

            with tc.tile_pool(name="ops", bufs=1, space="PSUM") as ops, \
                 tc.tile_pool(name="ow", bufs=4) as ow:
                opst = [ops.tile([128, 512], F32, tag=f"op{lt}", name=f"op{lt}")
                        for lt in range(4)]
                for h in range(NHG):
                    for lt in range(4):
                        mmr(opst[lt], hT2[:, h, lt * 128:(lt + 1) * 128],
                            wo2_sb[:, h, :], start=(h == 0), stop=(h == NHG - 1))
                for lt in range(4):
                    ob = ow.tile([128, 512], F32, tag="ob", name="ob")
                    nc.vector.tensor_tensor(ob, opst[lt], bo_sb, ADD)
                    nc.sync.dma_start(
                        out=out[lt * 128:(lt + 1) * 128, :], in_=ob)

    return nc


def get_nc():
    key = tuple(sorted(OPTS.items()))
    if key not in _NC_CACHE:
        nc = _build_nc()
        if not nc.is_finalized():
            nc.finalize()
        _NC_CACHE[key] = nc
    return _NC_CACHE[key]


# ---------------------------------------------------------------- host side
def make_in_maps(latents, target, target_mask, target_timestamp,
                 Wq, bq, Wk, bk, Wv, bv, Wo, bo):
    cos_tab, sin_tab = _host_tables()
    P2 = _rot_perm2()

    lat_ts = (np.arange(L, dtype=np.float32) * (MAX_POS - 1) / (L - 1)).astype(np.int64)
    cosq = np.tile(cos_tab[lat_ts].T, (2, 1)).astype(_BF)   # [128, L]
    sinq = np.tile(sin_tab[lat_ts].T, (2, 1)).astype(_BF)
    pt2 = np.ascontiguousarray(P2.T).astype(_BF)

    WoT = np.ascontiguousarray(np.asarray(Wo).T)            # [1024, 1024]

    # per-batch shared prep
    tgt8_b, cosk8_b, sink8_b, mb_b = [], [], [], []
    for b in range(B):
        tgtT = np.asarray(target[b]).T                      # [1024, T]
        tgt8_b.append(np.ascontiguousarray(
            tgtT.reshape(8, 128, NSL, 512).transpose(2, 1, 0, 3)
                .reshape(NSL, 128, T)).astype(_BF))
        ts = np.asarray(target_timestamp[b]).astype(np.int64)
        ck = np.tile(cos_tab[ts].T, (2, 1))                  # [128, T]
        sk = np.tile(sin_tab[ts].T, (2, 1))
        cosk8_b.append(np.ascontiguousarray(
            ck.reshape(128, NSL, 512).transpose(1, 0, 2)).astype(_BF))
        sink8_b.append(np.ascontiguousarray(
            sk.reshape(128, NSL, 512).transpose(1, 0, 2)).astype(_BF))
        mask = np.asarray(target_mask[b]).astype(np.float32)
        mb_b.append(np.ascontiguousarray(
            ((mask - 1.0) * 30000.0).reshape(NT, 128).T).astype(np.float32))

    latT = np.asarray(latents).transpose(0, 2, 1)           # [B, 1024, L]

    in_maps = []
    for core in range(NCORES):
        b, g = core // 2, core % 2
        sl = slice(g * C, (g + 1) * C)
        csl = slice(g * 512, (g + 1) * 512)                  # output col half
        wqT = np.asarray(Wq)[sl, :].T                        # [1024, C]
        wkT = np.asarray(Wk)[sl, :].T
        wvT = np.asarray(Wv)[sl, :].T
        # wo2: rows [g0 h d | g1 h d] fixed order, cols = this core's half
        wo2 = np.ascontiguousarray(
            WoT.reshape(2, NHG, 64, HID)[:, :, :, csl]
               .transpose(0, 2, 1, 3).reshape(128, NHG * 512)).astype(_BF)
        m = {
            "lat8": np.ascontiguousarray(
                latT[b].reshape(8, 128, L)).astype(_BF),
            "wq8": np.ascontiguousarray(wqT.reshape(8, 128, C)).astype(_BF),
            "bqw": np.ascontiguousarray(
                np.asarray(bq)[sl].reshape(C // 128, 128).T.astype(np.float32)),
            "cosq": cosq, "sinq": sinq, "pt2": pt2,
            "mbias": mb_b[b],
            "wk1": np.ascontiguousarray(
                wkT.reshape(8, 128, C).transpose(1, 0, 2)
                   .reshape(128, 8 * C)).astype(_BF),
            "bkw": np.ascontiguousarray(
                np.asarray(bk)[sl].reshape(C // 128, 128).T.astype(np.float32)),
            "tgt8": tgt8_b[b],
            "wv1": np.ascontiguousarray(
                wvT.reshape(8, 128, C).transpose(1, 0, 2)
                   .reshape(128, 8 * C)).astype(_BF),
            "bvrep": np.ascontiguousarray(
                np.tile(np.asarray(bv)[sl][None, :], (128, 1)).astype(np.float32)),
            "cosk8": cosk8_b[b], "sink8": sink8_b[b],
            "wo2": wo2,
            "borep": np.ascontiguousarray(
                np.tile(np.asarray(bo)[csl][None, :], (128, 1)).astype(np.float32)),
        }
        in_maps.append(m)
    return in_maps


def kernel(latents, target, target_mask, target_timestamp,
           Wq, bq, Wk, bk, Wv, bv, Wo, bo, _trace=False, _trace_kwargs=None):
    in_maps = make_in_maps(latents, target, target_mask, target_timestamp,
                           Wq, bq, Wk, bk, Wv, bv, Wo, bo)
    nc = get_nc()
    res = run_bass_kernel_spmd(nc, in_maps, list(range(NCORES)),
                               trace=_trace, **(_trace_kwargs or {}))
    full = np.zeros((B, L, HID), np.float32)
    for b in range(B):
        full[b][:, 0:512] = res.results[2 * b]["out"]
        full[b][:, 512:1024] = res.results[2 * b + 1]["out"]
    if _trace:
        return full, res
    return full
